# revision 115
# baseline (speedup 1.0000x reference)
"""DSConv (deformable "snake" conv block) Trainium2 Bass kernel.

Reference math (per batch b):
  off   = conv3x3(f) + off_b -> BN(eval) -> tanh ; x_off = channels 9:18
  cum   = cumulative offsets along k from center (matrix `tri`)
  X     = h + x_off_cum (sampling x-coord; y = w + k - 4 is always integer)
  samp[b,c,w,h,k] = (0<=y<=126 and 0<=X<127) ? lerp_x(f[b,c,y,:], X) : 0
  out   = snake conv: out[b,co,w,h] = sum_{ci,k} conv_w[co,ci,0,k] samp[b,ci,w,h,k]
  out   = GroupNorm(32 groups over (4co,W,H)) -> ReLU
Sharding: 8 cores = 2 batch x 4 W-quarters (32 output rows each).

Per-core pipeline (fp16 matmul operands, fp32 PSUM):
  offset  conv3x3 per w-chunk: dh=0/1 share one 64-wide matmul (partition
          groups 0:9, 32:41), dh=2 accumulates onto group 0 via an h+2 rhs
          window; groups summed by copy+add (conv phase is PE-bound, so
          trading PE columns for Act/DVE ops wins) -> tanh(+bvec);
          cum matmul [19,9] also folds the invalid-y BIG bias and a
          +(h-64) const row, so braw16 = h+cum-64 (+BIG).
  cliffs  the bilinear clip makes the reference discontinuous at X=0/127
          (only reachable at h in {0,1,2} u {125,126,127} since |cum|<3);
          those 10 fcv columns are recomputed with hi/lo-f16 conv and fp32
          cum so the mask decision matches the fp32 reference to ~1e-5.
          (-64 centering keeps the interior f16 X-rounding <= 0.031.)
  braw16  staged to DRAM in (wl,k,h) layout (quarters staged mid-conv so
          the main loop starts early).
  negA    per 4-wl block: ONE DMA broadcasts the contiguous DRAM slice to
          all 128 partitions (engines cannot partition-broadcast; DMA from
          DRAM can); then 3 DVE TSPs, all in 4x mode: -(x-64) per-partition
          scalar, |.| via sign-bit clear, fused min(.,1)-1. No PE, no PSUM.
  q       Q[x,(k,co)] = fs^T @ wt per y-row; k=0..7 into a [128,2,512] f32
          PSUM tile drained by ONE 1024-wide Act copy (matmul chunks may
          cross PSUM banks); k=8 accumulates in a shared 1-bank tile
          drained to a persistent qtail every 4 rows.
  step2   po[co,h] += q[wl+k][:,k,:]^T @ negA[:,k,:]; 4 wl accumulate in
          one PSUM bank, drained together (DVE) + bn_stats per block.
  GN      bn_stats -> raw per-core moments [128,2] -> AllGather (15us
          fixed cost, the dominant tail item) -> one matmul folds the
          group-sum with cores kept in the free axis, reduce over cores,
          sqrt+recip, expand matmul -> relu(S*x+B) split across Act
          (activation) and the post-collective-idle DVE (two fused 4x
          TSPs per block), f16 out.
Engine balance: Act and DVE both ~100% busy through the main loop (Act:
q drains + tanh + relu; DVE: negA chains + outC/bn + GN); PE ~78%. The
fs load rides the SWDGE (gpsimd) queue so braw staging/broadcast DMAs
aren't stuck behind it on the in-order SP HWDGE queue.
"""

import numpy as np
from contextlib import ExitStack

import concourse.bass as bass
import concourse.bacc as bacc
import concourse.tile as tile
import concourse.mybir as mybir
from concourse import bass_utils

F16 = mybir.dt.float16
F32 = mybir.dt.float32
U8 = mybir.dt.uint8
ALU = mybir.AluOpType
ACTF = mybir.ActivationFunctionType

K = 9
CENTER = 4
P = 128
W = 128
H = 128
B = 2
NY = 40            # sampling row window per core: y in [w0-4, w0+36)
WC = 32            # output w-rows per core
EPS = 1e-5
EXTEND = 1.0
BIG = 30000.0
NCORES = 8
NKH = K * P        # 1152
NPAIR = WC // 2    # 16 wl-pairs


def _tri_base():
    """tri[k, kp] = coeff of x_off[k] in cumulative offset new[kp]."""
    t = np.zeros((K, K), np.float32)
    t[0, 0] = 1.0
    t[K - 1, K - 1] = 1.0
    for i in range(1, CENTER):
        for j in range(CENTER + 1, CENTER + i + 1):
            t[j, CENTER + i] = 1.0
        for j in range(CENTER - i, CENTER):
            t[j, CENTER - i] = 1.0
    return t


def build_nc():
    nc = bacc.Bacc("TRN2", target_bir_lowering=False, debug=False,
                   num_devices=NCORES)

    fs_d = nc.dram_tensor("fs", [P, NY, P], F16, kind="ExternalInput")
    fcvh_d = nc.dram_tensor("fcvh", [P, 34, 130], F16, kind="ExternalInput")
    wt_d = nc.dram_tensor("wt", [P, NKH], F16, kind="ExternalInput")
    owh_d = nc.dram_tensor("owh", [P, 3, 96], F16, kind="ExternalInput")
    bvec_d = nc.dram_tensor("bvec", [K, 1], F32, kind="ExternalInput")
    triext_d = nc.dram_tensor("triext", [2 * K + 1, K], F16, kind="ExternalInput")
    ybt_d = nc.dram_tensor("ybt", [K, WC * P], F16, kind="ExternalInput")
    big32_d = nc.dram_tensor("big32", [K, 1], F32, kind="ExternalInput")
    owl_d = nc.dram_tensor("owl", [P, 3, 96], F16, kind="ExternalInput")
    triext32_d = nc.dram_tensor("triext32", [2 * K + 1, K], F32,
                                kind="ExternalInput")
    fcvbh_d = nc.dram_tensor("fcvbh", [P, 34, 10], F16, kind="ExternalInput")
    fcvbl_d = nc.dram_tensor("fcvbl", [P, 34, 10], F16, kind="ExternalInput")
    ybtb_d = nc.dram_tensor("ybtb", [K, WC * 6], F32, kind="ExternalInput")
    hrow_d = nc.dram_tensor("hrow", [1, WC * P], F16, kind="ExternalInput")
    hbrow_d = nc.dram_tensor("hbrow", [1, WC * 6], F32, kind="ExternalInput")
    xvec_d = nc.dram_tensor("xvec", [P, 1], F32, kind="ExternalInput")
    epsk_d = nc.dram_tensor("epsk", [32, 1], F32, kind="ExternalInput")
    gmat_d = nc.dram_tensor("gmat", [P, 32], F32, kind="ExternalInput")
    gexp_d = nc.dram_tensor("gexp", [32, P], F32, kind="ExternalInput")
    ngam_d = nc.dram_tensor("ngam", [P, 1], F32, kind="ExternalInput")
    bet_d = nc.dram_tensor("bet", [P, 1], F32, kind="ExternalInput")
    out_d = nc.dram_tensor("out", [P, WC, P], F16, kind="ExternalOutput")

    braw_d = nc.dram_tensor("braw_dram", [1, WC * NKH], F16, kind="Internal")
    cr_in = nc.dram_tensor("cr_in", [P, 2], F32, kind="Internal")
    cr_out = nc.dram_tensor("cr_out", [4 * P, 2], F32, kind="Internal")

    with tile.TileContext(nc) as tc, ExitStack() as ctx:
        const = ctx.enter_context(tc.tile_pool(name="const", bufs=1))
        mid = ctx.enter_context(tc.tile_pool(name="mid", bufs=1))

        def load(name, dram, shape, dtype):
            t = const.tile(shape, dtype, name=name + "_sb")
            nc.sync.dma_start(out=t, in_=dram.ap())
            return t

        owh = load("owh", owh_d, [P, 3, 96], F16)
        bvec = load("bvec", bvec_d, [K, 1], F32)
        triext = load("triext", triext_d, [2 * K + 1, K], F16)
        fcvh = const.tile([P, 34, 130], F16, name="fcvh_sb")
        for lo, hi in ((0, 6), (6, 14), (14, 24), (24, 34)):
            nc.sync.dma_start(out=fcvh[:, lo:hi, :],
                              in_=fcvh_d.ap()[:, lo:hi, :])
        xoff18 = const.tile([2 * K + 1, WC, P], F16, name="xoff18_sb")
        nc.sync.dma_start(
            out=xoff18.rearrange("p w h -> p (w h)")[K:2 * K, :],
            in_=ybt_d.ap())
        nc.sync.dma_start(
            out=xoff18.rearrange("p w h -> p (w h)")[2 * K:2 * K + 1, :],
            in_=hrow_d.ap())
        wt = load("wt", wt_d, [P, NKH], F16)
        fs = const.tile([P, NY, P], F16, name="fs_sb")
        for lo, hi in ((0, 10), (10, 24), (24, NY)):
            nc.gpsimd.dma_start(out=fs[:, lo:hi, :],
                                in_=fs_d.ap()[:, lo:hi, :])
        big32 = load("big32", big32_d, [K, 1], F32)
        owl = load("owl", owl_d, [P, 3, 96], F16)
        triext32 = load("triext32", triext32_d, [2 * K + 1, K], F32)
        fcvbh4 = load("fcvbh", fcvbh_d, [P, 34, 10], F16)
        fcvbl4 = load("fcvbl", fcvbl_d, [P, 34, 10], F16)
        fcvbh = fcvbh4.rearrange("p w (s c) -> p w s c", s=2)
        fcvbl = fcvbl4.rearrange("p w (s c) -> p w s c", s=2)
        xoffb32 = const.tile([2 * K + 1, WC, 2, 3], F32, name="xoffb32_sb")
        nc.sync.dma_start(
            out=xoffb32.rearrange("p w s c -> p (w s c)")[K:2 * K, :],
            in_=ybtb_d.ap())
        nc.sync.dma_start(
            out=xoffb32.rearrange("p w s c -> p (w s c)")[2 * K:2 * K + 1, :],
            in_=hbrow_d.ap())
        xvec = load("xvec", xvec_d, [P, 1], F32)
        epsk = load("epsk", epsk_d, [32, 1], F32)
        gmat = load("gmat", gmat_d, [P, 32], F32)
        gexp = load("gexp", gexp_d, [32, P], F32)
        ngam = load("ngam", ngam_d, [P, 1], F32)
        bet = load("bet", bet_d, [P, 1], F32)

        braw16 = mid.tile([K, WC, P], F16)
        bb32 = mid.tile([K, WC, 2, 3], F32)
        outC = mid.tile([P, WC, P], F16)
        stats = mid.tile([P, 9, 6], F32)
        qtail = mid.tile([P, NY, P], F16)   # k=8 slice of Q per y-row

        qtiles = {}
        qpool = ctx.enter_context(tc.tile_pool(name="qpool", bufs=13))
        napool = ctx.enter_context(tc.tile_pool(name="napool", bufs=6))
        fpool = ctx.enter_context(tc.tile_pool(name="fpool", bufs=4))
        psQ = ctx.enter_context(tc.tile_pool(name="psQ", bufs=2,
                                             space="PSUM"))
        psM = ctx.enter_context(tc.tile_pool(name="psM", bufs=1,
                                             space="PSUM"))

        # k=8 accumulates into one shared 1-bank PSUM tile, drained to the
        # persistent qtail every 4 rows by a single 512-wide copy.
        pstail = ctx.enter_context(tc.tile_pool(name="psT", bufs=1,
                                                space="PSUM"))
        pstail_t = pstail.tile([P, 4, P], F32, name="pstail")

        # q-drain engine schedule: per y-row, which engine drains the
        # [P,1024] PSUM read. Conv phase (rows 0-8) mixes (Act also has
        # tanh); main phase leans Act (DVE owns the negA chain).
        QENG = (["v", "v", "s", "v", "s", "v", "s", "v", "s"]
                + ["s"] * 33)

        def q_row(yl):
            q = qpool.tile([P, 8, P], F16, tag="q", name="q_sb")
            psq = psQ.tile([P, 2, 512], F32, tag="q", name="psq")
            for j in range(2):
                nc.tensor.matmul(psq[:, j, :],
                                 lhsT=fs[:, yl, :],
                                 rhs=wt[:, j * 512:(j + 1) * 512],
                                 start=True, stop=True)
            nc.tensor.matmul(pstail_t[:, yl % 4, :], lhsT=fs[:, yl, :],
                             rhs=wt[:, 1024:NKH], start=True, stop=True)
            qf = q.rearrange("p k c -> p (k c)")
            psqf = psq.rearrange("p a c -> p (a c)")
            if QENG[yl] == "v":
                nc.vector.tensor_copy(out=qf, in_=psqf)
            else:
                nc.scalar.copy(out=qf, in_=psqf)
            if yl % 4 == 3:
                tout = qtail[:, yl - 3:yl + 1, :].rearrange("p a c -> p (a c)")
                tin = pstail_t.rearrange("p a c -> p (a c)")
                if yl >= 36:
                    nc.vector.tensor_copy(out=tout, in_=tin)
                else:
                    nc.scalar.copy(out=tout, in_=tin)
            qtiles[yl] = q

        # DRAM staging of braw16 in (wl, k, h) layout: a per-pair slice is
        # one contiguous 4608B row that a single DMA can broadcast to all
        # 128 partitions.
        braw_dv = braw_d.ap().rearrange("o (w k h) -> o k w h", k=K, h=P)
        NBLK = WC // 4
        natiles = {}

        def issue_bcast(b):
            if b >= NBLK:
                return
            na4 = napool.tile([P, 4, K, P], F16, tag="na", name="na4")
            nc.sync.dma_start(
                out=na4.rearrange("p a k h -> p (a k h)"),
                in_=braw_d.ap()[0:1, b * 4 * NKH:(b + 1) * 4 * NKH]
                    .to_broadcast([P, 4 * NKH]))
            natiles[b] = na4

        def negA_chain(b):
            if b >= NBLK:
                return
            na4 = natiles[b]
            naf = na4.rearrange("p a k h -> p (a k h)")
            nc.vector.tensor_scalar(out=naf, in0=naf, scalar1=xvec,
                                    scalar2=None, op0=ALU.subtract)
            nau = naf.bitcast(mybir.dt.uint16)
            nc.vector.tensor_scalar(out=nau, in0=nau, scalar1=0x7FFF,
                                    scalar2=None, op0=ALU.bitwise_and)
            nc.vector.tensor_scalar(out=naf, in0=naf, scalar1=1.0,
                                    scalar2=1.0, op0=ALU.min,
                                    op1=ALU.subtract)

        # ---- offset branch (+ first 9 q rows interleaved) ----
        with tc.tile_pool(name="x9p", bufs=4) as x9p, \
             tc.tile_pool(name="psC", bufs=2, space="PSUM") as psC:
            def emit_boundary():
                # ---- precise boundary path ----
                # The bilinear clip makes the reference DISCONTINUOUS at X=0
                # and X=127, which (|cum|<3) can only hit output columns h in
                # {0,1,2} u {125,126,127}. Those samples' cum must match the
                # fp32 reference to ~1e-5 or a sample can land on the wrong
                # side of the cliff (O(1) error). Recompute just the 10
                # affected fcv columns with hi/lo f16 conv + fp32 cum, decide
                # the masks there, and overwrite braw16's 6 boundary columns.
                pssb = psC.tile([64, WC, 2, 5], F32, tag="cv", name="pssb")
                for dw in range(3):
                    rh = fcvbh[:, dw:dw + WC, :, :]
                    rl = fcvbl[:, dw:dw + WC, :, :]
                    rh2 = fcvbh[:, dw:dw + WC, :, 2:5]
                    rl2 = fcvbl[:, dw:dw + WC, :, 2:5]
                    nc.tensor.matmul(pssb, lhsT=owh[:, dw, 0:64], rhs=rh,
                                     start=(dw == 0), stop=False)
                    nc.tensor.matmul(pssb, lhsT=owl[:, dw, 0:64], rhs=rh,
                                     start=False, stop=False)
                    nc.tensor.matmul(pssb, lhsT=owh[:, dw, 0:64], rhs=rl,
                                     start=False, stop=False)
                    nc.tensor.matmul(pssb[0:K, :, :, 0:3],
                                     lhsT=owh[:, dw, 64:64 + K], rhs=rh2,
                                     start=False, stop=False)
                    nc.tensor.matmul(pssb[0:K, :, :, 0:3],
                                     lhsT=owl[:, dw, 64:64 + K], rhs=rh2,
                                     start=False, stop=False)
                    nc.tensor.matmul(pssb[0:K, :, :, 0:3],
                                     lhsT=owh[:, dw, 64:64 + K], rhs=rl2,
                                     start=False, stop=(dw == 2))
                g0b = x9p.tile([K, WC, 2, 3], F32, tag="g0", name="g0b")
                x9b = x9p.tile([K, WC, 2, 3], F32, tag="x9", name="x9b")
                nc.scalar.copy(out=g0b, in_=pssb[0:K, :, :, 0:3])
                nc.vector.tensor_tensor(out=x9b, in0=pssb[32:32 + K, :, :, 1:4],
                                        in1=g0b, op=ALU.add)
                nc.scalar.activation(out=xoffb32[0:K, :, :, :], in_=x9b,
                                     func=ACTF.Tanh, bias=bvec, scale=1.0)
                pcumb = psM.tile([K, WC, 2, 3], F32, tag="cm", name="pcumb")
                nc.tensor.matmul(pcumb, lhsT=triext32,
                                 rhs=xoffb32.rearrange("p w s c -> p (w s c)"),
                                 start=True, stop=True)
                nc.vector.tensor_copy(out=bb32, in_=pcumb)
                m1 = mid.tile([K, WC, 3], U8)
                m2 = mid.tile([K, WC, 3], U8)
                nc.vector.tensor_scalar(out=m1, in0=bb32[:, :, 0, :],
                                        scalar1=-64.0, scalar2=None,
                                        op0=ALU.is_lt)
                nc.vector.copy_predicated(
                    out=bb32[:, :, 0, :], mask=m1,
                    data=big32.to_broadcast([K, WC, 3]))
                nc.vector.tensor_scalar(out=m2, in0=bb32[:, :, 1, :],
                                        scalar1=63.0, scalar2=None,
                                        op0=ALU.is_ge)
                nc.vector.copy_predicated(
                    out=bb32[:, :, 1, :], mask=m2,
                    data=big32.to_broadcast([K, WC, 3]))
                nc.vector.tensor_copy(out=braw16[:, :, 0:3],
                                      in_=bb32[:, :, 0, :])
                nc.vector.tensor_copy(out=braw16[:, :, 125:128],
                                      in_=bb32[:, :, 1, :])

            wchunks = [(c3, min(3, WC - c3)) for c3 in range(0, WC, 3)]
            for ci_, (c3, nw) in enumerate(wchunks):
                # dh=0 and dh=1 in one 64-wide matmul (partition groups
                # 0:9 and 32:41), dh=2 accumulated onto group 0 via an h+2
                # rhs window; groups summed by copy+add (conv phase is
                # PE-bound, so trading PE columns for Act/DVE ops wins)
                pss = psC.tile([64, 3, 130], F32, tag="cv", name="pss")
                for dw in range(3):
                    nc.tensor.matmul(pss[:, :nw, :], lhsT=owh[:, dw, 0:64],
                                     rhs=fcvh[:, c3 + dw: c3 + dw + nw, :],
                                     start=(dw == 0), stop=False)
                    nc.tensor.matmul(pss[0:K, :nw, 0:128],
                                     lhsT=owh[:, dw, 64:64 + K],
                                     rhs=fcvh[:, c3 + dw: c3 + dw + nw, 2:130],
                                     start=False, stop=(dw == 2))
                g0 = x9p.tile([K, 3, P], F32, tag="g0", name="g0")
                x9t = x9p.tile([K, 3, P], F32, tag="x9", name="x9t")
                nc.scalar.copy(out=g0[:, :nw, :], in_=pss[0:K, :nw, 0:128])
                nc.vector.tensor_tensor(out=x9t[:, :nw, :],
                                        in0=pss[32:32 + K, :nw, 1:129],
                                        in1=g0[:, :nw, :], op=ALU.add)
                nc.scalar.activation(out=xoff18[0:K, c3:c3 + nw, :],
                                     in_=x9t[:, :nw, :], func=ACTF.Tanh,
                                     bias=bvec, scale=1.0)
                pcum = psM.tile([K, 3, P], F32, tag="cm", name="pcum")
                nc.tensor.matmul(pcum[:, :nw, :], lhsT=triext,
                                 rhs=xoff18[:, c3:c3 + nw, :],
                                 start=True, stop=True)
                if ci_ % 2 == 0:
                    nc.vector.tensor_copy(out=braw16[:, c3:c3 + nw, 3:125],
                                          in_=pcum[:, :nw, 3:125])
                else:
                    nc.scalar.copy(out=braw16[:, c3:c3 + nw, 3:125],
                                   in_=pcum[:, :nw, 3:125])
                if ci_ < K:
                    q_row(ci_)
                if ci_ == 1:
                    emit_boundary()
                if ci_ == 3:
                    # rows 0:8 of braw16 (incl. boundary cols) are complete:
                    # stage them; block-0's broadcast + negA chain runs on
                    # the conv-phase-idle DVE
                    nc.sync.dma_start(out=braw_dv[0, :, 0:8, :],
                                      in_=braw16[:, 0:8, :])
                    issue_bcast(0)
                    negA_chain(0)
                if ci_ == 6:
                    nc.sync.dma_start(out=braw_dv[0, :, 8:16, :],
                                      in_=braw16[:, 8:16, :])
                    issue_bcast(1)
                    negA_chain(1)

        # second DRAM half (first half staged mid-conv)
        nc.sync.dma_start(out=braw_dv[0, :, 16:WC, :], in_=braw16[:, 16:WC, :])

        # ---- main loop over 4-wl blocks ----
        with tc.tile_pool(name="poB", bufs=2, space="PSUM") as poB:
            for b in range(2, 5):
                issue_bcast(b)
            negA_chain(2)
            # run q production 3 rows ahead of step2 so the 4-batched qtail
            # drains land before their first consumer
            q_row(9)
            q_row(10)
            q_row(11)

            for blk in range(NBLK):
                issue_bcast(blk + 5)
                po = poB.tile([P, 4, P], F32, tag="po", name="po4")
                na4 = natiles.pop(blk)
                negA_chain(blk + 3)
                for j in range(4):
                    wl = 4 * blk + j
                    q_next = 12 + wl
                    if q_next < NY:
                        q_row(q_next)
                    for k in range(K):
                        lhsT = (qtiles[wl + k][:, k, :] if k < 8
                                else qtail[:, wl + 8, :])
                        nc.tensor.matmul(po[:, j, :],
                                         lhsT=lhsT,
                                         rhs=na4[:, j, k, :],
                                         start=(k == 0), stop=(k == K - 1))
                    del qtiles[wl]
                    if blk == NBLK - 1 and j % 2 == 1:
                        half = (j - 1) // 2
                        eng_c = nc.vector.tensor_copy if half else nc.scalar.copy
                        eng_c(
                            out=outC[:, WC - 4 + 2 * half:WC - 2 + 2 * half, :]
                                .rearrange("p a b -> p (a b)"),
                            in_=po[:, 2 * half:2 * half + 2, :]
                                .rearrange("p a b -> p (a b)"))
                        nc.vector.bn_stats(
                            out=stats[:, 7 + half, :],
                            in_=outC[:, WC - 4 + 2 * half:WC - 2 + 2 * half, :]
                                .rearrange("p a b -> p (a b)"))
                if blk < NBLK - 1:
                    nc.vector.tensor_copy(
                        out=outC[:, 4 * blk:4 * blk + 4, :]
                            .rearrange("p a b -> p (a b)"),
                        in_=po.rearrange("p a b -> p (a b)"))
                    nc.vector.bn_stats(
                        out=stats[:, blk, :],
                        in_=outC[:, 4 * blk:4 * blk + 4, :]
                            .rearrange("p a b -> p (a b)"))

            # ---- GroupNorm ----
            st2 = mid.tile([P, 2], F32)
            nc.vector.bn_aggr(out=st2, in_=stats)
            sq = mid.tile([P, 1], F32)
            nc.vector.tensor_tensor(out=sq, in0=st2[:, 0:1], in1=st2[:, 0:1],
                                    op=ALU.mult)
            nc.vector.tensor_tensor(out=st2[:, 1:2], in0=st2[:, 1:2], in1=sq,
                                    op=ALU.add)
            nc.sync.dma_start(out=cr_in.ap(), in_=st2)
            nc.gpsimd.collective_compute(
                kind="AllGather", op=ALU.bypass,
                replica_groups=[[0, 1, 2, 3], [4, 5, 6, 7]],
                ins=[cr_in.ap()], outs=[cr_out.ap()])
            gg = mid.tile([P, 4, 2], F32)
            nc.sync.dma_start(
                out=gg, in_=cr_out.ap().rearrange("(r c) v -> c r v", r=4))
            # group-sum across channels via matmul; cores stay in free axis
            pg = psM.tile([32, 4, 2], F32, tag="cm", name="ps_g")
            nc.tensor.matmul(pg, lhsT=gmat,
                             rhs=gg.rearrange("c r v -> c (r v)"),
                             start=True, stop=True)
            g2s = mid.tile([32, 2], F32)
            nc.vector.tensor_reduce(
                out=g2s, in_=pg.rearrange("c r v -> c v r"),
                axis=mybir.AxisListType.X, op=ALU.add)
            m2 = mid.tile([32, 1], F32)
            nc.vector.tensor_tensor(out=m2, in0=g2s[:, 0:1], in1=g2s[:, 0:1],
                                    op=ALU.mult)
            vg = mid.tile([32, 1], F32)
            nc.vector.tensor_tensor(out=vg, in0=g2s[:, 1:2], in1=m2,
                                    op=ALU.subtract)
            nc.scalar.activation(out=vg, in_=vg, func=ACTF.Sqrt,
                                 bias=epsk, scale=1.0)
            nc.vector.reciprocal(out=g2s[:, 1:2], in_=vg)
            pe2 = psM.tile([P, 2], F32, tag="cm", name="ps_e2")
            nc.tensor.matmul(pe2, lhsT=gexp, rhs=g2s, start=True, stop=True)
            Sv = mid.tile([P, 1], F32)
            nc.vector.tensor_tensor(out=Sv, in0=pe2[:, 1:2], in1=ngam,
                                    op=ALU.mult)
            t2 = mid.tile([P, 1], F32)
            nc.vector.tensor_tensor(out=t2, in0=pe2[:, 0:1], in1=Sv,
                                    op=ALU.mult)
            Bv = mid.tile([P, 1], F32)
            nc.vector.tensor_tensor(out=Bv, in0=bet, in1=t2, op=ALU.subtract)
            # final relu(S*x+B): split across Act (activation) and the
            # otherwise-idle DVE (two fused TSPs at 4x) so the out-DMA
            # stream starts sooner and drains faster
            blocks = ((0, 8, "v"), (8, 16, "s"), (16, 24, "v"),
                      (24, WC, "v"))
            for b0, b1, eng in blocks:
                fin = fpool.tile([P, b1 - b0, P], F16, tag="f", name="fin")
                if eng == "s":
                    nc.scalar.activation(out=fin, in_=outC[:, b0:b1, :],
                                         func=ACTF.Relu, bias=Bv, scale=Sv)
                else:
                    nc.vector.tensor_scalar(out=fin, in0=outC[:, b0:b1, :],
                                            scalar1=Sv, scalar2=None,
                                            op0=ALU.mult)
                    nc.vector.tensor_scalar(out=fin, in0=fin, scalar1=Bv,
                                            scalar2=0.0, op0=ALU.add,
                                            op1=ALU.max)
                nc.sync.dma_start(out=out_d.ap()[:, b0:b1, :], in_=fin)

    nc.compile()
    return nc


_TRI = _tri_base()


def prep_shared(off_w, off_b, bn_gamma, bn_beta, bn_mean, bn_var, conv_w,
                gn_gamma, gn_beta):
    s36 = (np.asarray(bn_gamma, np.float32)
           / np.sqrt(np.asarray(bn_var, np.float32) + EPS))
    s = s36[K:2 * K]
    bvec = ((np.asarray(off_b, np.float32)[K:2 * K]
             - np.asarray(bn_mean, np.float32)[K:2 * K]) * s
            + np.asarray(bn_beta, np.float32)[K:2 * K]
            ).reshape(K, 1).astype(np.float32)

    owf = np.asarray(off_w, np.float32)[K:2 * K]          # [k, ci, dw, dh]
    oww = np.zeros((P, 3, 96), np.float32)                # [ci, dw, (dh-group, k)]
    for dw in range(3):
        for dh in range(3):
            oww[:, dw, dh * 32: dh * 32 + K] = (owf[:, :, dw, dh] * s[:, None]).T
    owh = oww.astype(np.float16)
    owl = (oww - owh.astype(np.float32)).astype(np.float16)

    wtf = np.asarray(conv_w, np.float32)[:, :, 0, :]      # [co, ci, k]
    wt = np.ascontiguousarray(
        np.transpose(wtf, (1, 2, 0)).reshape(P, K * P)).astype(np.float16)

    hx = np.arange(P, dtype=np.float32)
    # braw rows carry z' = h + cum - 64 (centering halves the f16 ulp);
    # negA = min(|z' - (x-64)|, 1) - 1
    triext32 = np.concatenate([EXTEND * _TRI,
                               np.eye(K, dtype=np.float32),
                               np.ones((1, K), np.float32)], axis=0)
    triext = triext32.astype(np.float16)
    hrow = np.tile(hx - 64.0, WC).reshape(1, WC * P).astype(np.float16)
    hb = np.array([0.0, 1.0, 2.0, 125.0, 126.0, 127.0], np.float32) - 64.0
    hbrow = np.tile(hb, WC).reshape(1, WC * 6).astype(np.float32)
    return dict(
        wt=wt, owh=owh, owl=owl, bvec=bvec, triext=triext,
        triext32=triext32.astype(np.float32), hrow=hrow, hbrow=hbrow,
        xvec=(hx - 64.0).reshape(P, 1).astype(np.float32),
        epsk=np.full((32, 1), EPS, np.float32),
        big32=np.full((K, 1), BIG, np.float32),
        ngam=-np.asarray(gn_gamma, np.float32).reshape(P, 1),
        bet=np.asarray(gn_beta, np.float32).reshape(P, 1),
        gmat=np.array([[0.0625 if co // 4 == g else 0.0 for g in range(32)]
                       for co in range(P)], np.float32),
        gexp=np.array([[1.0 if co // 4 == g else 0.0 for co in range(P)]
                       for g in range(32)], np.float32),
    )


def prep_core(f, b, w0):
    fb = np.asarray(f, np.float32)[b]
    fs = np.zeros((P, NY, P), np.float16)
    lo = max(0, w0 - 4)
    hi = min(W, w0 + 36)
    fs[:, lo - (w0 - 4): hi - (w0 - 4), :] = fb[:, lo:hi, :].astype(np.float16)
    fpad = np.pad(fb, ((0, 0), (1, 1), (1, 1)))
    fcvh = np.ascontiguousarray(fpad[:, w0:w0 + 34, :]).astype(np.float16)
    b10 = [0, 1, 2, 3, 4, 125, 126, 127, 128, 129]
    fcvb = np.ascontiguousarray(fpad[:, w0:w0 + 34, :][:, :, b10])
    fcvbh = fcvb.astype(np.float16)
    fcvbl = (fcvb - fcvbh.astype(np.float32)).astype(np.float16)
    ybt = np.zeros((K, WC, P), np.float16)
    ybtb = np.zeros((K, WC, 6), np.float32)
    for wl in range(WC):
        for k in range(K):
            y = w0 + wl + k - 4
            if not (0 <= y <= 126):
                ybt[k, wl, :] = BIG
                ybtb[k, wl, :] = BIG
    return dict(fs=fs, fcvh=fcvh, ybt=ybt.reshape(K, WC * P),
                fcvbh=fcvbh, fcvbl=fcvbl, ybtb=ybtb.reshape(K, WC * 6))


_NC_CACHE = {}


def get_nc():
    if "nc" not in _NC_CACHE:
        _NC_CACHE["nc"] = build_nc()
    return _NC_CACHE["nc"]


def make_in_maps(f, off_w, off_b, bn_gamma, bn_beta, bn_mean, bn_var,
                 conv_w, conv_b, gn_gamma, gn_beta):
    consts = prep_shared(off_w, off_b, bn_gamma, bn_beta, bn_mean, bn_var,
                         conv_w, gn_gamma, gn_beta)
    in_maps = []
    for c in range(NCORES):
        b, q = c // 4, c % 4
        m = dict(consts)
        m.update(prep_core(f, b, q * WC))
        in_maps.append(m)
    return in_maps


def assemble(results):
    out = np.zeros((B, P, W, H), np.float32)
    for c in range(NCORES):
        b, q = c // 4, c % 4
        out[b, :, q * WC:(q + 1) * WC, :] = results[c]["out"].astype(np.float32)
    return out


def kernel(f, off_w, off_b, bn_gamma, bn_beta, bn_mean, bn_var,
           conv_w, conv_b, gn_gamma, gn_beta, **run_kwargs):
    nc = get_nc()
    in_maps = make_in_maps(f, off_w, off_b, bn_gamma, bn_beta, bn_mean,
                           bn_var, conv_w, conv_b, gn_gamma, gn_beta)
    last_exc = None
    for _attempt in range(3):
        try:
            res = bass_utils.run_bass_kernel_spmd(
                nc, in_maps, core_ids=list(range(NCORES)), **run_kwargs)
            break
        except Exception as e:  # transient tunnel/device hiccups
            last_exc = e
    else:
        raise last_exc
    out = assemble(res.results)
    kernel.last_result = res
    return out
